# revision 12
# baseline (speedup 1.0000x reference)
"""CapsuleLayer (dynamic routing, 3 iterations) Trainium2 Bass kernel.

Problem (hardcoded):
    x: [64, 2048, 8] f32, W: [2048, 32, 8, 16] f32
    u_hat[b,o,i,k] = sum_d x[b,i,d] * W[i,o,d,k]
    3 rounds of routing-by-agreement over logits b[B,O,I], softmax over O.
    out v: [64, 32, 16] f32.

Sharding: data-parallel over batch across 8 cores (8 batch rows each), W
replicated. Everything on-chip per core.

v2 layout/schedule:
  - pass 0: W DMA'd in 4-tile chunks (3-deep prefetch); per i-tile of 16:
    u_hat via block-diag matmul (xblk stationary, w streamed) + s0 via a
    second accumulating matmul (xt stationary). 6-deep PSUM rotation keeps
    PE back-to-back (p-state ramp); PSUM->SBUF bf16 casts rotate over
    DVE/ACT/GpSimd.
  - rounds 1,2 per group of 8 tiles: vu = u*v (DVE or GpSimd), k-tree
    reduce, logits; softmax as one exp (ACT) + reduce_sum + recip + scale
    (DVE); cu = u*c (DVE or GpSimd); s += ones-matmul over i-partitions.
  - squash + partition broadcast of v via PE ones-matmul.
Free-dim layout is (k, o): column = k*32 + o.
"""

import numpy as np
import ml_dtypes

BF16 = ml_dtypes.bfloat16

B, I, D, O, K = 64, 2048, 8, 32, 16
NC_N = 8              # cores
BL = B // NC_N        # 8 batch rows per core
G = 16                # i's per tile
T = I // G            # 128 tiles
FREE = O * K          # 512, layout (k,o): col = k*32+o
EPS = 1e-7
GB = 8                # tiles per round instruction group
NG = T // GB          # 16 groups
SUP = 4               # groups per softmax super-group
CH = 4                # tiles per w DMA chunk
NCH = T // CH         # 32 chunks

# engine split for round elementwise work (group index -> gpsimd)
VU_GPS = {1, 4, 7, 10, 13}
CU_GPS = {2, 5, 8, 11, 14}
L1_GPS = {6, 12}

_CACHE = {}


def _build_bass():
    import concourse.bass as bass
    import concourse.bacc as bacc
    import concourse.mybir as mybir
    import concourse.tile as tile

    f32 = mybir.dt.float32
    bf16 = mybir.dt.bfloat16
    nc = bacc.Bacc()

    wd = nc.dram_tensor("w", [NCH, 128, CH * FREE], bf16, kind="ExternalInput")
    xtd = nc.dram_tensor("xt", [128, T, BL], bf16, kind="ExternalInput")
    xblkd = nc.dram_tensor("xblk", [128, T, 128], bf16, kind="ExternalInput")
    onesd = nc.dram_tensor("ones", [128, BL], bf16, kind="ExternalInput")
    onestd = nc.dram_tensor("onest", [BL, 128], bf16, kind="ExternalInput")
    outd = nc.dram_tensor("out", [BL, FREE], f32, kind="ExternalOutput")

    AX = mybir.AxisListType
    ACTF = mybir.ActivationFunctionType

    with tile.TileContext(nc) as tc:
        with (
            tc.tile_pool(name="const", bufs=1) as constp,
            tc.tile_pool(name="u16", bufs=1) as up,
            tc.tile_pool(name="logits", bufs=1) as blp,
            tc.tile_pool(name="vexp", bufs=2) as vexpp,
            tc.tile_pool(name="psum_s", bufs=1, space="PSUM") as psum_s,
            tc.tile_pool(name="psum_v", bufs=1, space="PSUM") as psum_v,
        ):
            eps_sb = constp.tile([128, 1], f32)
            nc.gpsimd.memset(eps_sb[:], EPS)
            xt_sb = constp.tile([128, T, BL], bf16)
            nc.gpsimd.dma_start(xt_sb[:], xtd[:])
            ones_sb = constp.tile([128, BL], bf16)
            nc.gpsimd.dma_start(ones_sb[:], onesd[:])
            onest_sb = constp.tile([BL, 128], bf16)
            nc.gpsimd.dma_start(onest_sb[:], onestd[:])

            u16 = up.tile([128, T, FREE], bf16)
            bL = blp.tile([128, T, O], bf16)

            # ---------------- pass 0: u_hat + s0 ----------------
            s0_ps = psum_s.tile([BL, FREE], f32)
            with (
                tc.tile_pool(name="xblk", bufs=1) as xblkp,
                tc.tile_pool(name="wt", bufs=3) as wtp,
                tc.tile_pool(name="psum_u", bufs=5, space="PSUM") as psum_u,
            ):
                # block-diag x built host-side: xblk[g*8+d, t, g*8+b] = x[b, t*16+g, d]
                # 4 chunked tiles so the first matmuls don't wait on the
                # whole 4.2 MB transfer
                XCH = T // 4
                xblks = []
                for xc in range(4):
                    xb = xblkp.tile([128, XCH, 128], bf16, tag=f"xblk{xc}")
                    nc.gpsimd.dma_start(
                        xb[:], xblkd[:, xc * XCH:(xc + 1) * XCH, :])
                    xblks.append(xb)
                for c in range(NCH):
                    wt = wtp.tile([128, CH * FREE], bf16, tag="wt")
                    nc.gpsimd.dma_start(wt[:], wd[c])
                    for tt in range(CH):
                        t = c * CH + tt
                        ws = wt[:, tt * FREE:(tt + 1) * FREE]
                        ut_ps = psum_u.tile([128, FREE], f32, tag="ut")
                        nc.tensor.matmul(
                            ut_ps[:], xblks[t // XCH][:, t % XCH, :], ws)
                        # s0 accumulation straight from x,W
                        nc.tensor.matmul(
                            s0_ps[:], xt_sb[:, t, :], ws,
                            start=(t == 0), stop=(t == T - 1),
                        )
                        # PSUM -> SBUF bf16 cast copy, split DVE/ACT
                        # (GPSIMD cannot access PSUM)
                        if t % 2 == 0:
                            nc.vector.tensor_copy(u16[:, t, :], ut_ps[:])
                        else:
                            nc.scalar.copy(u16[:, t, :], ut_ps[:])

            # ---------------- squash + broadcast helper ----------------
            with tc.tile_pool(name="sq", bufs=1) as sqp:

                def squash_and_bcast(s_ps, scale_const, last):
                    """v = squash(s_ps * scale_const); returns vexp1 [128,FREE]
                    bf16 or DMAs fp32 v to outd if last."""
                    s = sqp.tile([BL, FREE], f32, tag="s")
                    nc.scalar.mul(s[:], s_ps[:], scale_const)
                    # s2[o] = sum_k s^2  (k stride is O in (k,o) layout)
                    sq2 = sqp.tile([BL, O, K], f32, tag="sq2")
                    nc.vector.tensor_mul(
                        sq2[:], s[:].rearrange("p (k o) -> p o k", o=O),
                        s[:].rearrange("p (k o) -> p o k", o=O))
                    s2 = sqp.tile([BL, O], f32, tag="s2")
                    nc.vector.reduce_sum(s2[:], sq2[:], axis=AX.X)
                    rt = sqp.tile([BL, O], f32, tag="rt")
                    nc.scalar.activation(rt[:], s2[:], ACTF.Sqrt, bias=eps_sb[:BL])
                    onep = sqp.tile([BL, O], f32, tag="onep")
                    nc.scalar.add(onep[:], s2[:], 1.0)
                    den = sqp.tile([BL, O], f32, tag="den")
                    nc.vector.tensor_mul(den[:], rt[:], onep[:])
                    rden = sqp.tile([BL, O], f32, tag="rden")
                    nc.vector.reciprocal(rden[:], den[:])
                    scl = sqp.tile([BL, O], f32, tag="scl")
                    nc.vector.tensor_mul(scl[:], s2[:], rden[:])
                    # v = s * scl (broadcast over k)
                    v = sqp.tile([BL, K, O], f32 if last else bf16, tag="v")
                    nc.vector.tensor_mul(
                        v[:], s[:].rearrange("p (k o) -> p k o", o=O),
                        scl[:].unsqueeze(1).broadcast_to([BL, K, O]))
                    if last:
                        nc.gpsimd.dma_start(outd[:], v[:].rearrange("p k o -> p (k o)"))
                        return None
                    # replicate v to all 16 partition groups via PE
                    vrep_ps = psum_v.tile([128, FREE], f32, tag="vrep")
                    nc.tensor.matmul(
                        vrep_ps[:], onest_sb[:],
                        v[:].rearrange("p k o -> p (k o)"))
                    vexp1 = vexpp.tile([128, FREE], bf16, tag="vexp1")
                    nc.scalar.copy(vexp1[:], vrep_ps[:])
                    return vexp1

                vexp1 = squash_and_bcast(s0_ps, 1.0 / O, last=False)

                # ---------------- rounds 1, 2 ----------------
                with (
                    tc.tile_pool(name="rnd", bufs=2) as rp,
                    tc.tile_pool(name="rnd1", bufs=1) as rp1,
                    tc.tile_pool(name="rnd2", bufs=2) as rp2,
                ):
                    for rnd in (1, 2):
                        s_ps = psum_s.tile([BL, FREE], f32, tag="s_ps")
                        vexp_b = vexp1[:].unsqueeze(1).broadcast_to(
                            [128, GB, FREE])
                        for g in range(NG):
                            gs, ge = g * GB, (g + 1) * GB
                            u_sl = u16[:, gs:ge, :]
                            e_vu = nc.gpsimd if g in VU_GPS else nc.vector
                            e_l1 = nc.gpsimd if g in L1_GPS else nc.vector
                            e_cu = nc.gpsimd if g in CU_GPS else nc.vector

                            vu = rp.tile([128, GB, FREE], bf16, tag="vu")
                            e_vu.tensor_mul(vu[:], u_sl, vexp_b)
                            # k-tree reduce (k outer, stride O)
                            vuv = vu[:].rearrange("p t (k o) -> p t k o", o=O)
                            t1 = rp.tile([128, GB, 8, O], bf16, tag="t1")
                            e_l1.tensor_add(t1[:], vuv[:, :, 0:8], vuv[:, :, 8:16])
                            t2 = rp1.tile([128, GB, 4, O], bf16, tag="t2")
                            nc.vector.tensor_add(t2[:], t1[:, :, 0:4], t1[:, :, 4:8])
                            t3 = rp1.tile([128, GB, 2, O], bf16, tag="t3")
                            nc.vector.tensor_add(t3[:], t2[:, :, 0:2], t2[:, :, 2:4])
                            if rnd == 1:
                                # logits = agreement (b starts at 0)
                                nc.vector.tensor_add(
                                    bL[:, gs:ge, :], t3[:, :, 0, :], t3[:, :, 1, :])
                                lg = bL[:, gs:ge, :]
                            else:
                                agr = rp2.tile([128, GB, O], bf16, tag="agr")
                                nc.vector.tensor_add(
                                    agr[:], t3[:, :, 0, :], t3[:, :, 1, :])
                                lg2 = rp2.tile([128, GB, O], bf16, tag="lg2")
                                nc.vector.tensor_add(
                                    lg2[:], agr[:], bL[:, gs:ge, :])
                                lg = lg2[:]
                            # exp per group into the super e-buffer
                            si = g // SUP        # super index
                            gi = g % SUP         # group within super
                            if gi == 0:
                                e_sup = rp2.tile(
                                    [128, SUP * GB, O], bf16, tag="e_sup")
                                cu_src = rp2.tile(
                                    [128, SUP * GB, O], bf16, tag="c_sup")
                            nc.scalar.activation(
                                e_sup[:, gi * GB:(gi + 1) * GB, :], lg, ACTF.Exp)
                            if gi == SUP - 1:
                                # softmax normalize for the whole super-group
                                z = rp2.tile([128, SUP * GB], f32, tag="z")
                                nc.vector.reduce_sum(z[:], e_sup[:], axis=AX.X)
                                rz = rp2.tile([128, SUP * GB], f32, tag="rz")
                                nc.vector.reciprocal(rz[:], z[:])
                                nc.vector.tensor_mul(
                                    cu_src[:], e_sup[:],
                                    rz[:].unsqueeze(2).broadcast_to(
                                        [128, SUP * GB, O]))
                                # cu + s-matmuls for the 4 groups of this super
                                for g2 in range(si * SUP, (si + 1) * SUP):
                                    g2s = g2 * GB
                                    e2 = nc.gpsimd if g2 in CU_GPS else nc.vector
                                    cu = rp.tile([128, GB, K, O], bf16, tag="cu")
                                    e2.tensor_mul(
                                        cu[:],
                                        u16[:, g2s:g2s + GB, :].rearrange(
                                            "p t (k o) -> p t k o", o=O),
                                        cu_src[:, (g2 - si * SUP) * GB:
                                               (g2 - si * SUP + 1) * GB, :]
                                        .unsqueeze(2).broadcast_to(
                                            [128, GB, K, O]))
                                    for j in range(GB):
                                        t = g2s + j
                                        nc.tensor.matmul(
                                            s_ps[:], ones_sb[:],
                                            cu[:, j, :, :].rearrange(
                                                "p k o -> p (k o)"),
                                            start=(t == 0), stop=(t == T - 1))
                        vexp1 = squash_and_bcast(s_ps, 1.0, last=(rnd == 2))
    nc.finalize()
    return nc


def _host_prep():
    """Core-independent input prep pieces."""
    ones = np.zeros((128, BL), dtype=BF16)
    for g in range(G):
        for b in range(BL):
            ones[g * 8 + b, b] = 1
    onest = np.ascontiguousarray(ones.T)
    return ones, onest


def kernel(x: np.ndarray, W: np.ndarray) -> np.ndarray:
    from concourse import bass_utils

    if "nc" not in _CACHE:
        _CACHE["nc"] = _build_bass()
        _CACHE["ones"], _CACHE["onest"] = _host_prep()
    nc = _CACHE["nc"]

    # W -> [T, (g,d), (k,o)] : w[t, g*8+d, k*32+o] = W[t*16+g, o, d, k]
    wr = (W.reshape(T, G, O, D, K).transpose(0, 1, 3, 4, 2)
          .reshape(T, 128, FREE).astype(BF16))
    # chunk 4 tiles per DMA: [NCH, 128, CH*FREE]
    wch = np.ascontiguousarray(
        wr.reshape(NCH, CH, 128, FREE).transpose(0, 2, 1, 3)
        .reshape(NCH, 128, CH * FREE))
    in_maps = []
    for c in range(NC_N):
        xl = x[c * BL:(c + 1) * BL]  # [8, 2048, 8]
        # xt[g*8+d, t, b] = xl[b, t*16+g, d]
        xt = np.ascontiguousarray(
            xl.reshape(BL, T, G, D).transpose(2, 3, 1, 0).reshape(128, T, BL)
        ).astype(BF16)
        xblk = np.zeros((128, T, 128), dtype=BF16)
        for g in range(G):
            xblk[g * 8:(g + 1) * 8, :, g * 8:(g + 1) * 8] = xt[g * 8:(g + 1) * 8]
        in_maps.append({"w": wch, "xt": xt, "xblk": xblk, "ones": _CACHE["ones"],
                        "onest": _CACHE["onest"]})

    _CACHE["in_maps"] = in_maps
    res = bass_utils.run_bass_kernel_spmd(nc, in_maps, core_ids=list(range(NC_N)))
    out = np.empty((B, O, K), np.float32)
    for c in range(NC_N):
        v = res.results[c]["out"].reshape(BL, K, O)  # (k,o) cols
        out[c * BL:(c + 1) * BL] = v.transpose(0, 2, 1)
    return out


# revision 13
# speedup vs baseline: 1.3254x; 1.3254x over previous
"""CapsuleLayer (dynamic routing, 3 iterations) Trainium2 Bass kernel.

Problem (hardcoded):
    x: [64, 2048, 8] f32, W: [2048, 32, 8, 16] f32
    u_hat[b,o,i,k] = sum_d x[b,i,d] * W[i,o,d,k]
    3 rounds of routing-by-agreement over logits b[B,O,I], softmax over O.
    out v: [64, 32, 16] f32.

Sharding: data-parallel over batch across 8 cores (8 batch rows each), W
replicated. Everything on-chip per core.

v2 layout/schedule:
  - pass 0: W DMA'd in 4-tile chunks (3-deep prefetch); per i-tile of 16:
    u_hat via block-diag matmul (xblk stationary, w streamed) + s0 via a
    second accumulating matmul (xt stationary). 6-deep PSUM rotation keeps
    PE back-to-back (p-state ramp); PSUM->SBUF bf16 casts rotate over
    DVE/ACT/GpSimd.
  - rounds 1,2 per group of 8 tiles: vu = u*v (DVE or GpSimd), k-tree
    reduce, logits; softmax as one exp (ACT) + reduce_sum + recip + scale
    (DVE); cu = u*c (DVE or GpSimd); s += ones-matmul over i-partitions.
  - squash + partition broadcast of v via PE ones-matmul.
Free-dim layout is (k, o): column = k*32 + o.
"""

import numpy as np
import ml_dtypes

BF16 = ml_dtypes.bfloat16

B, I, D, O, K = 64, 2048, 8, 32, 16
NC_N = 8              # cores
BL = B // NC_N        # 8 batch rows per core
G = 16                # i's per tile
T = I // G            # 128 tiles
FREE = O * K          # 512, layout (k,o): col = k*32+o
EPS = 1e-7
GB = 8                # tiles per round instruction group
NG = T // GB          # 16 groups
SUP = 4               # groups per softmax super-group
CH = 4                # tiles per w DMA chunk
NCH = T // CH         # 32 chunks

# engine split for round elementwise work (group index -> gpsimd).
# gpsimd TT measured ~2.4 ns/elem (4.3x slower than DVE 2x) and pool-slot
# rotation chains DVE ops behind slow gps ops -> keep everything on DVE.
VU_GPS = set()
CU_GPS = set()
L1_GPS = set()

_CACHE = {}


def _build_bass():
    import concourse.bass as bass
    import concourse.bacc as bacc
    import concourse.mybir as mybir
    import concourse.tile as tile

    f32 = mybir.dt.float32
    bf16 = mybir.dt.bfloat16
    nc = bacc.Bacc()

    wd = nc.dram_tensor("w", [NCH, 128, CH * FREE], bf16, kind="ExternalInput")
    xtd = nc.dram_tensor("xt", [128, T, BL], bf16, kind="ExternalInput")
    xblkd = nc.dram_tensor("xblk", [128, T, 128], bf16, kind="ExternalInput")
    onesd = nc.dram_tensor("ones", [128, BL], bf16, kind="ExternalInput")
    onestd = nc.dram_tensor("onest", [BL, 128], bf16, kind="ExternalInput")
    outd = nc.dram_tensor("out", [BL, FREE], f32, kind="ExternalOutput")

    AX = mybir.AxisListType
    ACTF = mybir.ActivationFunctionType

    with tile.TileContext(nc) as tc:
        with (
            tc.tile_pool(name="const", bufs=1) as constp,
            tc.tile_pool(name="u16", bufs=1) as up,
            tc.tile_pool(name="logits", bufs=1) as blp,
            tc.tile_pool(name="vexp", bufs=2) as vexpp,
            tc.tile_pool(name="psum_s", bufs=1, space="PSUM") as psum_s,
            tc.tile_pool(name="psum_v", bufs=1, space="PSUM") as psum_v,
        ):
            eps_sb = constp.tile([128, 1], f32)
            nc.gpsimd.memset(eps_sb[:], EPS)
            xt_sb = constp.tile([128, T, BL], bf16)
            nc.gpsimd.dma_start(xt_sb[:], xtd[:])
            ones_sb = constp.tile([128, BL], bf16)
            nc.gpsimd.dma_start(ones_sb[:], onesd[:])
            onest_sb = constp.tile([BL, 128], bf16)
            nc.gpsimd.dma_start(onest_sb[:], onestd[:])

            u16 = up.tile([128, T, FREE], bf16)
            bL = blp.tile([128, T, O], bf16)

            # ---------------- pass 0: u_hat + s0 ----------------
            s0_ps = psum_s.tile([BL, FREE], f32)
            with (
                tc.tile_pool(name="xblk", bufs=1) as xblkp,
                tc.tile_pool(name="wt", bufs=3) as wtp,
                tc.tile_pool(name="psum_u", bufs=5, space="PSUM") as psum_u,
            ):
                # block-diag x built host-side: xblk[g*8+d, t, g*8+b] = x[b, t*16+g, d]
                # 4 chunked tiles so the first matmuls don't wait on the
                # whole 4.2 MB transfer
                XCH = T // 4
                xblks = []
                for xc in range(4):
                    xb = xblkp.tile([128, XCH, 128], bf16, tag=f"xblk{xc}")
                    nc.gpsimd.dma_start(
                        xb[:], xblkd[:, xc * XCH:(xc + 1) * XCH, :])
                    xblks.append(xb)
                for c in range(NCH):
                    wt = wtp.tile([128, CH * FREE], bf16, tag="wt")
                    nc.gpsimd.dma_start(wt[:], wd[c])
                    for tt in range(CH):
                        t = c * CH + tt
                        ws = wt[:, tt * FREE:(tt + 1) * FREE]
                        ut_ps = psum_u.tile([128, FREE], f32, tag="ut")
                        nc.tensor.matmul(
                            ut_ps[:], xblks[t // XCH][:, t % XCH, :], ws)
                        # s0 accumulation straight from x,W
                        nc.tensor.matmul(
                            s0_ps[:], xt_sb[:, t, :], ws,
                            start=(t == 0), stop=(t == T - 1),
                        )
                        # PSUM -> SBUF bf16 cast copy, split DVE/ACT
                        # (GPSIMD cannot access PSUM)
                        if t % 2 == 0:
                            nc.vector.tensor_copy(u16[:, t, :], ut_ps[:])
                        else:
                            nc.scalar.copy(u16[:, t, :], ut_ps[:])

            # ---------------- squash + broadcast helper ----------------
            with tc.tile_pool(name="sq", bufs=1) as sqp:

                def squash_and_bcast(s_ps, scale_const, last):
                    """v = squash(s_ps * scale_const); returns vexp1 [128,FREE]
                    bf16 or DMAs fp32 v to outd if last."""
                    s = sqp.tile([BL, FREE], f32, tag="s")
                    nc.scalar.mul(s[:], s_ps[:], scale_const)
                    # s2[o] = sum_k s^2  (k stride is O in (k,o) layout)
                    sq2 = sqp.tile([BL, O, K], f32, tag="sq2")
                    nc.vector.tensor_mul(
                        sq2[:], s[:].rearrange("p (k o) -> p o k", o=O),
                        s[:].rearrange("p (k o) -> p o k", o=O))
                    s2 = sqp.tile([BL, O], f32, tag="s2")
                    nc.vector.reduce_sum(s2[:], sq2[:], axis=AX.X)
                    rt = sqp.tile([BL, O], f32, tag="rt")
                    nc.scalar.activation(rt[:], s2[:], ACTF.Sqrt, bias=eps_sb[:BL])
                    onep = sqp.tile([BL, O], f32, tag="onep")
                    nc.scalar.add(onep[:], s2[:], 1.0)
                    den = sqp.tile([BL, O], f32, tag="den")
                    nc.vector.tensor_mul(den[:], rt[:], onep[:])
                    rden = sqp.tile([BL, O], f32, tag="rden")
                    nc.vector.reciprocal(rden[:], den[:])
                    scl = sqp.tile([BL, O], f32, tag="scl")
                    nc.vector.tensor_mul(scl[:], s2[:], rden[:])
                    # v = s * scl (broadcast over k)
                    v = sqp.tile([BL, K, O], f32 if last else bf16, tag="v")
                    nc.vector.tensor_mul(
                        v[:], s[:].rearrange("p (k o) -> p k o", o=O),
                        scl[:].unsqueeze(1).broadcast_to([BL, K, O]))
                    if last:
                        nc.gpsimd.dma_start(outd[:], v[:].rearrange("p k o -> p (k o)"))
                        return None
                    # replicate v to all 16 partition groups via PE
                    vrep_ps = psum_v.tile([128, FREE], f32, tag="vrep")
                    nc.tensor.matmul(
                        vrep_ps[:], onest_sb[:],
                        v[:].rearrange("p k o -> p (k o)"))
                    vexp1 = vexpp.tile([128, FREE], bf16, tag="vexp1")
                    nc.scalar.copy(vexp1[:], vrep_ps[:])
                    return vexp1

                vexp1 = squash_and_bcast(s0_ps, 1.0 / O, last=False)

                # ---------------- rounds 1, 2 ----------------
                with (
                    tc.tile_pool(name="rnd", bufs=2) as rp,
                    tc.tile_pool(name="rnd1", bufs=1) as rp1,
                    tc.tile_pool(name="rnd2", bufs=2) as rp2,
                ):
                    for rnd in (1, 2):
                        s_ps = psum_s.tile([BL, FREE], f32, tag="s_ps")
                        vexp_b = vexp1[:].unsqueeze(1).broadcast_to(
                            [128, GB, FREE])
                        for g in range(NG):
                            gs, ge = g * GB, (g + 1) * GB
                            u_sl = u16[:, gs:ge, :]
                            e_vu = nc.gpsimd if g in VU_GPS else nc.vector
                            e_l1 = nc.gpsimd if g in L1_GPS else nc.vector
                            e_cu = nc.gpsimd if g in CU_GPS else nc.vector

                            vu = rp.tile([128, GB, FREE], bf16, tag="vu")
                            e_vu.tensor_mul(vu[:], u_sl, vexp_b)
                            # k-tree reduce (k outer, stride O)
                            vuv = vu[:].rearrange("p t (k o) -> p t k o", o=O)
                            t1 = rp.tile([128, GB, 8, O], bf16, tag="t1")
                            e_l1.tensor_add(t1[:], vuv[:, :, 0:8], vuv[:, :, 8:16])
                            t2 = rp1.tile([128, GB, 4, O], bf16, tag="t2")
                            nc.vector.tensor_add(t2[:], t1[:, :, 0:4], t1[:, :, 4:8])
                            t3 = rp1.tile([128, GB, 2, O], bf16, tag="t3")
                            nc.vector.tensor_add(t3[:], t2[:, :, 0:2], t2[:, :, 2:4])
                            if rnd == 1:
                                # logits = agreement (b starts at 0)
                                nc.vector.tensor_add(
                                    bL[:, gs:ge, :], t3[:, :, 0, :], t3[:, :, 1, :])
                                lg = bL[:, gs:ge, :]
                            else:
                                agr = rp2.tile([128, GB, O], bf16, tag="agr")
                                nc.vector.tensor_add(
                                    agr[:], t3[:, :, 0, :], t3[:, :, 1, :])
                                lg2 = rp2.tile([128, GB, O], bf16, tag="lg2")
                                nc.vector.tensor_add(
                                    lg2[:], agr[:], bL[:, gs:ge, :])
                                lg = lg2[:]
                            # exp per group into the super e-buffer
                            si = g // SUP        # super index
                            gi = g % SUP         # group within super
                            if gi == 0:
                                e_sup = rp2.tile(
                                    [128, SUP * GB, O], bf16, tag="e_sup")
                                cu_src = rp2.tile(
                                    [128, SUP * GB, O], bf16, tag="c_sup")
                            nc.scalar.activation(
                                e_sup[:, gi * GB:(gi + 1) * GB, :], lg, ACTF.Exp)
                            if gi == SUP - 1:
                                # softmax normalize for the whole super-group
                                z = rp2.tile([128, SUP * GB], f32, tag="z")
                                nc.vector.reduce_sum(z[:], e_sup[:], axis=AX.X)
                                rz = rp2.tile([128, SUP * GB], f32, tag="rz")
                                nc.vector.reciprocal(rz[:], z[:])
                                nc.vector.tensor_mul(
                                    cu_src[:], e_sup[:],
                                    rz[:].unsqueeze(2).broadcast_to(
                                        [128, SUP * GB, O]))
                                # cu + s-matmuls for the 4 groups of this super
                                for g2 in range(si * SUP, (si + 1) * SUP):
                                    g2s = g2 * GB
                                    e2 = nc.gpsimd if g2 in CU_GPS else nc.vector
                                    cu = rp.tile([128, GB, K, O], bf16, tag="cu")
                                    e2.tensor_mul(
                                        cu[:],
                                        u16[:, g2s:g2s + GB, :].rearrange(
                                            "p t (k o) -> p t k o", o=O),
                                        cu_src[:, (g2 - si * SUP) * GB:
                                               (g2 - si * SUP + 1) * GB, :]
                                        .unsqueeze(2).broadcast_to(
                                            [128, GB, K, O]))
                                    for j in range(GB):
                                        t = g2s + j
                                        nc.tensor.matmul(
                                            s_ps[:], ones_sb[:],
                                            cu[:, j, :, :].rearrange(
                                                "p k o -> p (k o)"),
                                            start=(t == 0), stop=(t == T - 1))
                        vexp1 = squash_and_bcast(s_ps, 1.0, last=(rnd == 2))
    nc.finalize()
    return nc


def _host_prep():
    """Core-independent input prep pieces."""
    ones = np.zeros((128, BL), dtype=BF16)
    for g in range(G):
        for b in range(BL):
            ones[g * 8 + b, b] = 1
    onest = np.ascontiguousarray(ones.T)
    return ones, onest


def kernel(x: np.ndarray, W: np.ndarray) -> np.ndarray:
    from concourse import bass_utils

    if "nc" not in _CACHE:
        _CACHE["nc"] = _build_bass()
        _CACHE["ones"], _CACHE["onest"] = _host_prep()
    nc = _CACHE["nc"]

    # W -> [T, (g,d), (k,o)] : w[t, g*8+d, k*32+o] = W[t*16+g, o, d, k]
    wr = (W.reshape(T, G, O, D, K).transpose(0, 1, 3, 4, 2)
          .reshape(T, 128, FREE).astype(BF16))
    # chunk 4 tiles per DMA: [NCH, 128, CH*FREE]
    wch = np.ascontiguousarray(
        wr.reshape(NCH, CH, 128, FREE).transpose(0, 2, 1, 3)
        .reshape(NCH, 128, CH * FREE))
    in_maps = []
    for c in range(NC_N):
        xl = x[c * BL:(c + 1) * BL]  # [8, 2048, 8]
        # xt[g*8+d, t, b] = xl[b, t*16+g, d]
        xt = np.ascontiguousarray(
            xl.reshape(BL, T, G, D).transpose(2, 3, 1, 0).reshape(128, T, BL)
        ).astype(BF16)
        xblk = np.zeros((128, T, 128), dtype=BF16)
        for g in range(G):
            xblk[g * 8:(g + 1) * 8, :, g * 8:(g + 1) * 8] = xt[g * 8:(g + 1) * 8]
        in_maps.append({"w": wch, "xt": xt, "xblk": xblk, "ones": _CACHE["ones"],
                        "onest": _CACHE["onest"]})

    _CACHE["in_maps"] = in_maps
    res = bass_utils.run_bass_kernel_spmd(nc, in_maps, core_ids=list(range(NC_N)))
    out = np.empty((B, O, K), np.float32)
    for c in range(NC_N):
        v = res.results[c]["out"].reshape(BL, K, O)  # (k,o) cols
        out[c * BL:(c + 1) * BL] = v.transpose(0, 2, 1)
    return out
